# revision 17
# baseline (speedup 1.0000x reference)
"""Trainium2 Bass kernel for a transformer block (LN -> causal MHA -> FFN).

Sharding (8 NeuronCores, one chip):
  - LayerNorm/RMSNorm: sequence-sharded (256 tokens/core), then AllGather of
    the transposed normed activations h^T (bf16) so every core holds full-seq h^T.
  - Attention: head-parallel (3 of 24 heads per core, full sequence, causal,
    no-max-subtraction softmax with the 1/rowsum deferred into a PE broadcast).
  - AllToAll converts head-sharded attention output o^T into sequence-sharded
    all-heads o^T; each core then computes Wo + residual, and the FFN
    (gelu(y@W1+b1)@W2+b2+y) for only its own 256 tokens with full (replicated,
    streamed) W1/W2. Final output is gathered on host from the 8 row-shards.

Matmuls run in bf16 with fp32 PSUM accumulation; norms, residuals and all
reductions stay fp32.
"""

import sys

for _p in ("/opt/trn_rl_repo",):
    if _p not in sys.path:
        sys.path.append(_p)

import numpy as np
import ml_dtypes

import concourse.bass as bass
import concourse.mybir as mybir
import concourse.tile as tile
from concourse import bacc
from concourse.bass_utils import run_bass_kernel_spmd
from concourse.masks import make_identity

AF = mybir.ActivationFunctionType
ALU = mybir.AluOpType

S, D, H, Dh, F = 2048, 2048, 24, 128, 8192
N_CORES = 8
S_LOC = S // N_CORES          # 256 tokens per core
H_LOC = H // N_CORES          # 3 heads per core
CW = H_LOC * Dh               # 384 qkv columns per core
SCALE = Dh ** -0.5
EPS = 1e-5

bf16 = mybir.dt.bfloat16
f32 = mybir.dt.float32
f32r = mybir.dt.float32r

TRACE = False        # test.py flips this for profiled runs
_CACHE = {}


def _emit(nc, tc, io):
    rg = [list(range(N_CORES))]
    x_r, lng, lnb, b2b, b1t, wqkv, wo, w1, w2, msk, onc, onr, out_r = io

    dram = tc.alloc_tile_pool(name="dram", bufs=1, space="DRAM")
    constp = tc.alloc_tile_pool(name="const", bufs=1)

    ag_in = dram.tile([D, S_LOC], bf16)
    ag_out = dram.tile([N_CORES * D, S_LOC], bf16, addr_space="Shared")
    a2a_in = dram.tile([N_CORES * CW, S_LOC], bf16)
    a2a_out = dram.tile([N_CORES * CW, S_LOC], bf16)

    # constants
    ident = constp.tile([128, 128], bf16)
    make_identity(nc, ident[:, :])
    lng_sb = constp.tile([128, D], bf16)
    nc.sync.dma_start(lng_sb[:, :], lng[:, :])
    lnb_sb = constp.tile([128, D], bf16)
    nc.sync.dma_start(lnb_sb[:, :], lnb[:, :])
    b2b_sb = constp.tile([128, D], bf16)
    nc.sync.dma_start(b2b_sb[:, :], b2b[:, :])
    b1t_sb = constp.tile([128, F // 128], f32)
    nc.sync.dma_start(b1t_sb[:, :], b1t[:, :])
    msk_sb = constp.tile([128, 2048], bf16)
    nc.sync.dma_start(msk_sb[:, :], msk[:, :])
    onc_sb = constp.tile([128, 1], bf16)
    nc.sync.dma_start(onc_sb[:, :], onc[:, :])
    onr_sb = constp.tile([1, 128], f32r)
    nc.sync.dma_start(onr_sb[:, :], onr[:, :])
    eps_sb = constp.tile([128, 1], f32)
    nc.vector.memset(eps_sb[:, :], EPS)

    # persistent activations (whole-kernel lifetime)
    persist = tc.alloc_tile_pool(name="persist", bufs=1)
    xln = [persist.tile([128, D], f32, name=f"xln{i}") for i in range(2)]
    y_sb = [persist.tile([128, D], f32, name=f"y{i}") for i in range(2)]

    # ---------------- Phase A: LN + RMSNorm + transpose (own tokens) -------
    with tc.tile_pool(name="phA", bufs=2) as sbA, \
         tc.tile_pool(name="phA_ps", bufs=2, space="PSUM") as psA:
        hT = [sbA.tile([128, S_LOC], bf16, name=f"hT{i}", bufs=1) for i in range(16)]
        for st in range(2):
            xa = sbA.tile([128, D], f32, tag="xa")
            nc.sync.dma_start(xa[:, :], x_r[st * 128:(st + 1) * 128, :])
            stats = sbA.tile([128, 24], f32, tag="stats")
            for a in range(4):
                nc.vector.bn_stats(stats[:, a * 6:(a + 1) * 6],
                                   xa[:, a * 512:(a + 1) * 512])
            aggr = sbA.tile([128, 2], f32, tag="aggr")
            nc.vector.bn_aggr(aggr[:, :], stats[:, :].rearrange("p (a b) -> p a b", b=6))
            std = sbA.tile([128, 1], f32, tag="std")
            nc.scalar.activation(std[:, :], aggr[:, 1:2], AF.Sqrt, bias=eps_sb[:, :])
            istd = sbA.tile([128, 1], f32, tag="istd")
            nc.vector.reciprocal(istd[:, :], std[:, :])
            nc.vector.tensor_scalar(
                out=xln[st][:, :], in0=xa[:, :],
                scalar1=aggr[:, 0:1], scalar2=istd[:, :],
                op0=ALU.subtract, op1=ALU.mult,
            )
            nc.vector.tensor_tensor(xln[st][:, :], xln[st][:, :], lng_sb[:, :], op=ALU.mult)
            nc.vector.tensor_tensor(xln[st][:, :], xln[st][:, :], lnb_sb[:, :], op=ALU.add)
            # rms stats of x_ln
            stats2 = sbA.tile([128, 24], f32, tag="stats2")
            for a in range(4):
                nc.vector.bn_stats(stats2[:, a * 6:(a + 1) * 6],
                                   xln[st][:, a * 512:(a + 1) * 512])
            aggr2 = sbA.tile([128, 2], f32, tag="aggr2")
            nc.vector.bn_aggr(aggr2[:, :], stats2[:, :].rearrange("p (a b) -> p a b", b=6))
            ms = sbA.tile([128, 1], f32, tag="ms")
            nc.vector.tensor_mul(ms[:, :], aggr2[:, 0:1], aggr2[:, 0:1])
            nc.vector.tensor_tensor(ms[:, :], ms[:, :], aggr2[:, 1:2], op=ALU.add)
            rstd = sbA.tile([128, 1], f32, tag="rstd")
            nc.scalar.activation(rstd[:, :], ms[:, :], AF.Sqrt, bias=eps_sb[:, :])
            irms = sbA.tile([128, 1], f32, tag="irms")
            nc.vector.reciprocal(irms[:, :], rstd[:, :])
            h = sbA.tile([128, D], bf16, tag="h")
            nc.vector.tensor_scalar(
                out=h[:, :], in0=xln[st][:, :],
                scalar1=irms[:, :], scalar2=None, op0=ALU.mult,
            )
            for dc in range(16):
                tp = psA.tile([128, 128], bf16, tag="tp")
                nc.tensor.transpose(tp[:, :], h[:, dc * 128:(dc + 1) * 128], ident[:, :])
                nc.vector.tensor_copy(hT[dc][:, st * 128:(st + 1) * 128], tp[:, :])
        for dc in range(16):
            nc.sync.dma_start(ag_in[dc * 128:(dc + 1) * 128, :], hT[dc][:, :])

    # ---------------- Phase B: AllGather h^T ------------------------------
    nc.gpsimd.collective_compute(
        "AllGather", ALU.bypass, replica_groups=rg,
        ins=[ag_in.opt()], outs=[ag_out.opt()],
    )

    # ---------------- Phase C: QKV projections ----------------------------
    pCD = tc.alloc_tile_pool(name="pCD", bufs=1)     # lives through phase D
    qkT = [pCD.tile([128, S], bf16, name=f"qkT{i}") for i in range(6)]
    vsb = [pCD.tile([128, CW], bf16, name=f"v{i}") for i in range(16)]
    with tc.tile_pool(name="phC_w", bufs=1) as wp, \
         tc.tile_pool(name="phC_h", bufs=1) as hp, \
         tc.tile_pool(name="phC_ps", bufs=2, space="PSUM") as psC:
        wq_sb = [wp.tile([128, 3 * CW], bf16, name=f"wqkv{i}") for i in range(16)]
        for dc in range(16):
            nc.sync.dma_start(wq_sb[dc][:, :], wqkv[dc * 128:(dc + 1) * 128, :])
        hTb = [hp.tile([128, S], bf16, name=f"hTb{i}") for i in range(16)]
        for dc in range(16):
            for r in range(N_CORES):
                nc.sync.dma_start(
                    hTb[dc][:, r * S_LOC:(r + 1) * S_LOC],
                    ag_out[r * D + dc * 128: r * D + (dc + 1) * 128, :],
                )
        # v first (direct [token, feature] layout), so attention can start early
        for stv in range(16):
            ps = psC.tile([128, CW], f32, tag="v_ps")
            for dc in range(16):
                nc.tensor.matmul(
                    ps[:, :],
                    lhsT=hTb[dc][:, stv * 128:(stv + 1) * 128],
                    rhs=wq_sb[dc][:, 2 * CW:3 * CW],
                    start=(dc == 0), stop=(dc == 15),
                )
            nc.any.tensor_copy(vsb[stv][:, :], ps[:, :])
        # q^T, k^T per head (transposed [feature, token] layout)
        for hh in range(3):
            for ct in (hh, 3 + hh):            # q-tile then k-tile of head hh
                for snb in range(8):
                    ps = psC.tile([128, S_LOC], f32, tag="qk_ps")
                    for dc in range(16):
                        nc.tensor.matmul(
                            ps[:, :],
                            lhsT=wq_sb[dc][:, ct * 128:(ct + 1) * 128],
                            rhs=hTb[dc][:, snb * S_LOC:(snb + 1) * S_LOC],
                            start=(dc == 0), stop=(dc == 15),
                        )
                    nc.any.tensor_copy(qkT[ct][:, snb * S_LOC:(snb + 1) * S_LOC], ps[:, :])

    # ---------------- Phase D: causal attention (3 heads, all tokens) ------
    with tc.tile_pool(name="phD", bufs=3) as sbD, \
         tc.tile_pool(name="phD_s", bufs=2, space="PSUM") as psDs, \
         tc.tile_pool(name="phD_o", bufs=2, space="PSUM") as psDo, \
         tc.tile_pool(name="phD_r", bufs=1, space="PSUM") as psDr, \
         tc.tile_pool(name="phD_b", bufs=1, space="PSUM") as psDb:
        for hh in range(3):
            qT = qkT[hh]
            kT = qkT[3 + hh]
            for qi in range(4):
                o_ps = psDo.tile([128, 512], f32, tag="o")
                r_ps = psDr.tile([1, 512], f32, tag="r")
                nk = 4 * (qi + 1)
                for ki in range(nk):
                    s_ps = psDs.tile([128, 512], f32, tag="s")
                    nc.tensor.matmul(
                        s_ps[:, :],
                        lhsT=kT[:, ki * 128:(ki + 1) * 128],
                        rhs=qT[:, qi * 512:(qi + 1) * 512],
                        start=True, stop=True,
                    )
                    p_sb = sbD.tile([128, 512], bf16, tag="p")
                    nc.scalar.activation(p_sb[:, :], s_ps[:, :], AF.Exp, scale=SCALE)
                    if ki >= 4 * qi:
                        m = ki - 4 * qi
                        nc.vector.tensor_tensor(
                            p_sb[:, :], p_sb[:, :], msk_sb[:, m * 512:(m + 1) * 512],
                            op=ALU.mult,
                        )
                    nc.tensor.matmul(
                        o_ps[:, :],
                        lhsT=vsb[ki][:, hh * 128:(hh + 1) * 128],
                        rhs=p_sb[:, :],
                        start=(ki == 0), stop=(ki == nk - 1),
                    )
                    nc.tensor.matmul(
                        r_ps[:, :],
                        lhsT=onc_sb[:, :],
                        rhs=p_sb[:, :],
                        start=(ki == 0), stop=(ki == nk - 1),
                    )
                rc = sbD.tile([1, 512], f32r, tag="rc")
                with nc.allow_low_precision(reason="f32r broadcast of softmax recip"):
                    nc.vector.reciprocal(rc[:, :], r_ps[:, :])
                bc_ps = psDb.tile([128, 512], f32, tag="bc")
                nc.tensor.matmul(
                    bc_ps[:, :], lhsT=onr_sb[:, :],
                    rhs=rc[:, :], start=True, stop=True,
                )
                bc_sb = sbD.tile([128, 512], f32, tag="bcs")
                nc.vector.tensor_copy(bc_sb[:, :], bc_ps[:, :])
                on_sb = sbD.tile([128, 512], bf16, tag="on")
                nc.vector.tensor_mul(on_sb[:, :], o_ps[:, :], bc_sb[:, :])
                for half in range(2):
                    j = 2 * qi + half
                    nc.sync.dma_start(
                        a2a_in[j * CW + hh * 128: j * CW + (hh + 1) * 128, :],
                        on_sb[:, half * 256:(half + 1) * 256],
                    )

    pCD.release()

    # ---------------- Phase E: AllToAll + Wo + residual --------------------
    nc.gpsimd.collective_compute(
        "AllToAll", ALU.bypass, replica_groups=rg,
        ins=[a2a_in.opt()], outs=[a2a_out.opt()],
    )
    with tc.tile_pool(name="phE", bufs=1) as sbE, \
         tc.tile_pool(name="phE_w", bufs=4) as sbEw, \
         tc.tile_pool(name="phE_ps", bufs=8, space="PSUM") as psE:
        oT = [sbE.tile([128, S_LOC], bf16, name=f"oT{i}") for i in range(24)]
        for cc in range(24):
            nc.sync.dma_start(oT[cc][:, :], a2a_out[cc * 128:(cc + 1) * 128, :])
        y_ps = [psE.tile([128, 512], f32, name=f"y_ps{i}", tag="y") for i in range(8)]
        for cc in range(24):
            wot = sbEw.tile([128, D], bf16, tag="wo")
            nc.sync.dma_start(wot[:, :], wo[cc * 128:(cc + 1) * 128, :])
            for st2 in range(2):
                for dt4 in range(4):
                    nc.tensor.matmul(
                        y_ps[st2 * 4 + dt4][:, :],
                        lhsT=oT[cc][:, st2 * 128:(st2 + 1) * 128],
                        rhs=wot[:, dt4 * 512:(dt4 + 1) * 512],
                        start=(cc == 0), stop=(cc == 23),
                    )
        for st2 in range(2):
            for dt4 in range(4):
                nc.vector.scalar_tensor_tensor(
                    out=y_sb[st2][:, dt4 * 512:(dt4 + 1) * 512],
                    in0=y_ps[st2 * 4 + dt4][:, :], scalar=1.0,
                    in1=xln[st2][:, dt4 * 512:(dt4 + 1) * 512],
                    op0=ALU.mult, op1=ALU.add,
                )

    # ---------------- Phase F: y -> y^T ------------------------------------
    pFG = tc.alloc_tile_pool(name="pFG", bufs=1)
    yT = [pFG.tile([128, S_LOC], bf16, name=f"yT{i}") for i in range(16)]
    with tc.tile_pool(name="phF", bufs=2) as sbF, \
         tc.tile_pool(name="phF_ps", bufs=2, space="PSUM") as psF:
        for st2 in range(2):
            ybf = sbF.tile([128, D], bf16, tag="ybf")
            nc.vector.tensor_copy(ybf[:, :], y_sb[st2][:, :])
            for dc in range(16):
                tp = psF.tile([128, 128], bf16, tag="tpf")
                nc.tensor.transpose(tp[:, :], ybf[:, dc * 128:(dc + 1) * 128], ident[:, :])
                nc.vector.tensor_copy(yT[dc][:, st2 * 128:(st2 + 1) * 128], tp[:, :])

    # ---------------- Phase G: FFN1 (gelu(y@W1+b1), transposed out) --------
    pGH = tc.alloc_tile_pool(name="pGH", bufs=1)
    gT = [pGH.tile([128, S_LOC], bf16, name=f"gT{i}") for i in range(64)]
    with tc.tile_pool(name="phG_w", bufs=32) as wg, \
         tc.tile_pool(name="phG_ps", bufs=2, space="PSUM") as psG:
        for fg in range(16):
            w1t = []
            for dc in range(16):
                t = wg.tile([128, 512], bf16, tag="w1")
                nc.sync.dma_start(
                    t[:, :], w1[dc * 128:(dc + 1) * 128, fg * 512:(fg + 1) * 512])
                w1t.append(t)
            for ft in range(4):
                g_ps = psG.tile([128, S_LOC], f32, tag="g")
                for dc in range(16):
                    nc.tensor.matmul(
                        g_ps[:, :],
                        lhsT=w1t[dc][:, ft * 128:(ft + 1) * 128],
                        rhs=yT[dc][:, :],
                        start=(dc == 0), stop=(dc == 15),
                    )
                fi = fg * 4 + ft
                nc.scalar.activation(
                    gT[fi][:, :], g_ps[:, :], AF.Gelu_apprx_tanh,
                    bias=b1t_sb[:, fi:fi + 1], scale=1.0,
                )

    # ---------------- Phase H: FFN2 + bias + residual ----------------------
    with tc.tile_pool(name="phH_w", bufs=8) as wh, \
         tc.tile_pool(name="phH_sb", bufs=2) as sbH, \
         tc.tile_pool(name="phH_ps", bufs=4, space="PSUM") as psH:
        for dt4 in range(4):
            z_ps = [psH.tile([128, 512], f32, tag="z", name=f"z{dt4}_{i}") for i in range(2)]
            for fc in range(64):
                w2t = wh.tile([128, 512], bf16, tag="w2")
                nc.sync.dma_start(
                    w2t[:, :], w2[fc * 128:(fc + 1) * 128, dt4 * 512:(dt4 + 1) * 512])
                for st2 in range(2):
                    nc.tensor.matmul(
                        z_ps[st2][:, :],
                        lhsT=gT[fc][:, st2 * 128:(st2 + 1) * 128],
                        rhs=w2t[:, :],
                        start=(fc == 0), stop=(fc == 63),
                    )
            for st2 in range(2):
                zt = sbH.tile([128, 512], f32, tag="zt")
                nc.vector.scalar_tensor_tensor(
                    out=zt[:, :], in0=z_ps[st2][:, :], scalar=1.0,
                    in1=y_sb[st2][:, dt4 * 512:(dt4 + 1) * 512],
                    op0=ALU.mult, op1=ALU.add,
                )
                nc.vector.tensor_tensor(
                    zt[:, :], zt[:, :], b2b_sb[:, dt4 * 512:(dt4 + 1) * 512], op=ALU.add)
                nc.sync.dma_start(
                    out_r[st2 * 128:(st2 + 1) * 128, dt4 * 512:(dt4 + 1) * 512], zt[:, :])

    pGH.release()
    pFG.release()
    persist.release()
    constp.release()
    dram.release()


def _build():
    if "nc" in _CACHE:
        return _CACHE["nc"]
    nc = bacc.Bacc("TRN2", target_bir_lowering=False, debug=False,
                   num_devices=N_CORES)

    def I(name, shape, dt):
        return nc.dram_tensor(name, shape, dt, kind="ExternalInput")

    io = (
        I("x_r", [S_LOC, D], f32),
        I("lng", [128, D], bf16),
        I("lnb", [128, D], bf16),
        I("b2b", [128, D], bf16),
        I("b1t", [128, F // 128], f32),
        I("wqkv", [D, 3 * CW], bf16),
        I("wo", [H * Dh, D], bf16),
        I("w1", [D, F], bf16),
        I("w2", [F, D], bf16),
        I("msk", [128, 2048], bf16),
        I("onc", [128, 1], bf16),
        I("onr", [1, 128], f32r),
        nc.dram_tensor("out_r", [S_LOC, D], f32, kind="ExternalOutput"),
    )
    with tile.TileContext(nc) as tc:
        _emit(nc, tc, io)
    nc.compile()
    _CACHE["nc"] = nc
    return nc


def _host_prep(inputs):
    bf = ml_dtypes.bfloat16
    x = np.asarray(inputs["x"], np.float32).reshape(S, D)
    ln_g = np.asarray(inputs["ln_g"], np.float32)
    ln_b = np.asarray(inputs["ln_b"], np.float32)
    attn_g = np.asarray(inputs["attn_g"], np.float32)
    Wq = np.asarray(inputs["Wq"], np.float32)
    Wk = np.asarray(inputs["Wk"], np.float32)
    Wv = np.asarray(inputs["Wv"], np.float32)
    Wo = np.asarray(inputs["Wo"], np.float32)
    W1 = np.asarray(inputs["W1"], np.float32)
    b1 = np.asarray(inputs["b1"], np.float32)
    W2 = np.asarray(inputs["W2"], np.float32)
    b2 = np.asarray(inputs["b2"], np.float32)

    g = attn_g[:, None]
    Wq_s = (Wq * g).astype(bf)
    Wk_s = (Wk * g).astype(bf)
    Wv_s = (Wv * g).astype(bf)
    wo_b = Wo.astype(bf)
    w1_b = W1.astype(bf)
    w2_b = W2.astype(bf)

    lng_b = np.broadcast_to(ln_g[None, :], (128, D)).astype(bf)
    lnb_b = np.broadcast_to(ln_b[None, :], (128, D)).astype(bf)
    b2_b = np.broadcast_to(b2[None, :], (128, D)).astype(bf)
    b1_t = np.ascontiguousarray(b1.reshape(F // 128, 128).T).astype(np.float32)

    i_idx = np.arange(512)[None, :]
    j_idx = np.arange(128)[:, None]
    msk = np.concatenate(
        [(i_idx >= 128 * m + j_idx) for m in range(4)], axis=1
    ).astype(bf)
    onc = np.ones((128, 1), bf)
    onr = np.ones((1, 128), np.float32)

    in_maps = []
    for r in range(N_CORES):
        wqkv_r = np.concatenate(
            [Wq_s[:, r * CW:(r + 1) * CW],
             Wk_s[:, r * CW:(r + 1) * CW],
             Wv_s[:, r * CW:(r + 1) * CW]], axis=1)
        in_maps.append({
            "x_r": np.ascontiguousarray(x[r * S_LOC:(r + 1) * S_LOC, :]),
            "lng": lng_b, "lnb": lnb_b, "b2b": b2_b, "b1t": b1_t,
            "wqkv": np.ascontiguousarray(wqkv_r),
            "wo": wo_b, "w1": w1_b, "w2": w2_b,
            "msk": msk, "onc": onc, "onr": onr,
        })
    return in_maps


def kernel(**inputs) -> np.ndarray:
    nc = _build()
    in_maps = _host_prep(inputs)
    res = run_bass_kernel_spmd(
        nc, in_maps, core_ids=list(range(N_CORES)), trace=TRACE)
    _CACHE["last_result"] = res
    out = np.concatenate([res.results[r]["out_r"] for r in range(N_CORES)], axis=0)
    return out.reshape(1, S, D)


# revision 19
# speedup vs baseline: 1.1108x; 1.1108x over previous
"""Trainium2 Bass kernel for a transformer block (LN -> causal MHA -> FFN).

Sharding (8 NeuronCores, one chip):
  - LayerNorm/RMSNorm: sequence-sharded (256 tokens/core), then AllGather of
    the transposed normed activations h^T (bf16) so every core holds full-seq h^T.
  - Attention: head-parallel (3 of 24 heads per core, full sequence, causal,
    no-max-subtraction softmax with the 1/rowsum deferred into a PE broadcast).
  - AllToAll converts head-sharded attention output o^T into sequence-sharded
    all-heads o^T; each core then computes Wo + residual, and the FFN
    (gelu(y@W1+b1)@W2+b2+y) for only its own 256 tokens with full (replicated,
    streamed) W1/W2. Final output is gathered on host from the 8 row-shards.

Matmuls run in bf16 with fp32 PSUM accumulation; norms, residuals and all
reductions stay fp32.
"""

import sys

for _p in ("/opt/trn_rl_repo",):
    if _p not in sys.path:
        sys.path.append(_p)

import numpy as np
import ml_dtypes

import concourse.bass as bass
import concourse.mybir as mybir
import concourse.tile as tile
from concourse import bacc
from concourse.bass_utils import run_bass_kernel_spmd
from concourse.masks import make_identity

AF = mybir.ActivationFunctionType
ALU = mybir.AluOpType

S, D, H, Dh, F = 2048, 2048, 24, 128, 8192
N_CORES = 8
S_LOC = S // N_CORES          # 256 tokens per core
H_LOC = H // N_CORES          # 3 heads per core
CW = H_LOC * Dh               # 384 qkv columns per core
SCALE = Dh ** -0.5
EPS = 1e-5

bf16 = mybir.dt.bfloat16
f32 = mybir.dt.float32
f32r = mybir.dt.float32r

TRACE = False        # test.py flips this for profiled runs
_CACHE = {}


def _emit(nc, tc, io):
    rg = [list(range(N_CORES))]
    x_r, lng, lnb, b2b, b1t, wqkv, wo, w1, w2, msk, onc, onr, out_r = io

    dram = tc.alloc_tile_pool(name="dram", bufs=1, space="DRAM")
    constp = tc.alloc_tile_pool(name="const", bufs=1)

    ag_in = [dram.tile([D // 2, S_LOC], bf16, name=f"ag_in{i}") for i in range(2)]
    ag_out = [dram.tile([N_CORES * D // 2, S_LOC], bf16, addr_space="Shared",
                        name=f"ag_out{i}") for i in range(2)]
    a2a_in = dram.tile([N_CORES * CW, S_LOC], bf16)
    a2a_out = dram.tile([N_CORES * CW, S_LOC], bf16)

    # constants
    ident = constp.tile([128, 128], bf16)
    make_identity(nc, ident[:, :])
    lng_sb = constp.tile([128, D], bf16)
    nc.sync.dma_start(lng_sb[:, :], lng[:, :])
    lnb_sb = constp.tile([128, D], bf16)
    nc.sync.dma_start(lnb_sb[:, :], lnb[:, :])
    b2b_sb = constp.tile([128, D], bf16)
    nc.sync.dma_start(b2b_sb[:, :], b2b[:, :])
    b1t_sb = constp.tile([128, F // 128], f32)
    nc.sync.dma_start(b1t_sb[:, :], b1t[:, :])
    msk_sb = constp.tile([128, 2048], bf16)
    nc.sync.dma_start(msk_sb[:, :], msk[:, :])
    onc_sb = constp.tile([128, 1], bf16)
    nc.sync.dma_start(onc_sb[:, :], onc[:, :])
    onr_sb = constp.tile([1, 128], f32r)
    nc.sync.dma_start(onr_sb[:, :], onr[:, :])
    eps_sb = constp.tile([128, 1], f32)
    nc.vector.memset(eps_sb[:, :], EPS)

    # persistent activations (whole-kernel lifetime)
    persist = tc.alloc_tile_pool(name="persist", bufs=1)
    xln = [persist.tile([128, D], f32, name=f"xln{i}") for i in range(2)]
    y_sb = [persist.tile([128, D], f32, name=f"y{i}") for i in range(2)]

    # ---------------- Phase A: LN + RMSNorm + transpose (own tokens) -------
    with tc.tile_pool(name="phA", bufs=2) as sbA, \
         tc.tile_pool(name="phA_ps", bufs=4, space="PSUM") as psA:
        hT = sbA.tile([128, 16 * S_LOC], bf16, name="hT", bufs=1)
        hh_t = []
        for st in range(2):
            xa = sbA.tile([128, D], f32, tag="xa")
            nc.sync.dma_start(xa[:, :], x_r[st * 128:(st + 1) * 128, :])
            stats = sbA.tile([128, 24], f32, tag="stats")
            for a in range(4):
                nc.vector.bn_stats(stats[:, a * 6:(a + 1) * 6],
                                   xa[:, a * 512:(a + 1) * 512])
            aggr = sbA.tile([128, 2], f32, tag="aggr")
            nc.vector.bn_aggr(aggr[:, :], stats[:, :].rearrange("p (a b) -> p a b", b=6))
            std = sbA.tile([128, 1], f32, tag="std")
            nc.scalar.activation(std[:, :], aggr[:, 1:2], AF.Sqrt, bias=eps_sb[:, :])
            istd = sbA.tile([128, 1], f32, tag="istd")
            nc.vector.reciprocal(istd[:, :], std[:, :])
            nc.vector.tensor_scalar(
                out=xln[st][:, :], in0=xa[:, :],
                scalar1=aggr[:, 0:1], scalar2=istd[:, :],
                op0=ALU.subtract, op1=ALU.mult,
            )
            nc.vector.tensor_tensor(xln[st][:, :], xln[st][:, :], lng_sb[:, :], op=ALU.mult)
            nc.vector.tensor_tensor(xln[st][:, :], xln[st][:, :], lnb_sb[:, :], op=ALU.add)
            # rms stats of x_ln
            stats2 = sbA.tile([128, 24], f32, tag="stats2")
            for a in range(4):
                nc.vector.bn_stats(stats2[:, a * 6:(a + 1) * 6],
                                   xln[st][:, a * 512:(a + 1) * 512])
            aggr2 = sbA.tile([128, 2], f32, tag="aggr2")
            nc.vector.bn_aggr(aggr2[:, :], stats2[:, :].rearrange("p (a b) -> p a b", b=6))
            ms = sbA.tile([128, 1], f32, tag="ms")
            nc.vector.tensor_mul(ms[:, :], aggr2[:, 0:1], aggr2[:, 0:1])
            nc.vector.tensor_tensor(ms[:, :], ms[:, :], aggr2[:, 1:2], op=ALU.add)
            rstd = sbA.tile([128, 1], f32, tag="rstd")
            nc.scalar.activation(rstd[:, :], ms[:, :], AF.Sqrt, bias=eps_sb[:, :])
            irms = sbA.tile([128, 1], f32, tag="irms")
            nc.vector.reciprocal(irms[:, :], rstd[:, :])
            h = sbA.tile([128, D], bf16, tag="h")
            nc.vector.tensor_scalar(
                out=h[:, :], in0=xln[st][:, :],
                scalar1=irms[:, :], scalar2=None, op0=ALU.mult,
            )
            hh_t.append(h)
        # transpose: half 0 (d 0..1023) first, feed AG half 0 early
        for half in range(2):
            for dc in range(half * 8, half * 8 + 8):
                for st in range(2):
                    tp = psA.tile([128, 128], bf16, tag="tp")
                    nc.tensor.transpose(
                        tp[:, :], hh_t[st][:, dc * 128:(dc + 1) * 128], ident[:, :])
                    nc.vector.tensor_copy(
                        hT[:, dc * S_LOC + st * 128: dc * S_LOC + (st + 1) * 128],
                        tp[:, :])
            src = hT[:, half * 8 * S_LOC:(half + 1) * 8 * S_LOC]
            nc.sync.dma_start(
                ag_in[half][:, :].rearrange("(dc p) j -> p dc j", p=128),
                src.rearrange("p (dc j) -> p dc j", j=S_LOC),
            )
            # -------- Phase B: AllGather h^T (split in two halves) ---------
            nc.gpsimd.collective_compute(
                "AllGather", ALU.bypass, replica_groups=rg,
                ins=[ag_in[half].opt()], outs=[ag_out[half].opt()],
            )

    # ---------------- Phase C: QKV projections ----------------------------
    pCD = tc.alloc_tile_pool(name="pCD", bufs=1)     # lives through phase D
    qkT = [pCD.tile([128, S], bf16, name=f"qkT{i}") for i in range(6)]
    vsb = [pCD.tile([128, CW], bf16, name=f"v{i}") for i in range(16)]
    with tc.tile_pool(name="phC_w", bufs=1) as wp, \
         tc.tile_pool(name="phC_h", bufs=1) as hp, \
         tc.tile_pool(name="phC_ps", bufs=2, space="PSUM") as psC:
        wq_sb = [wp.tile([128, 4 * 3 * CW], bf16, name=f"wqkv{i}") for i in range(4)]
        for g4 in range(4):
            nc.sync.dma_start(
                wq_sb[g4][:, :].rearrange("p (dc c) -> p dc c", dc=4),
                wqkv[:, :].rearrange("(g dc p) c -> g p dc c", g=4, p=128)[g4],
            )
        wq = [wq_sb[dc // 4][:, (dc % 4) * 3 * CW:(dc % 4 + 1) * 3 * CW]
              for dc in range(16)]
        hTb = [hp.tile([128, S], bf16, name=f"hTb{i}") for i in range(16)]
        for dc in range(16):
            half, dl = dc // 8, dc % 8
            nc.sync.dma_start(
                hTb[dc][:, :].rearrange("p (r j) -> p r j", r=8),
                ag_out[half][:, :].rearrange(
                    "(r q p) j -> q p r j", r=8, p=128)[dl],
            )
        # v first (direct [token, feature] layout), so attention can start early
        for stv in range(16):
            ps = psC.tile([128, CW], f32, tag="v_ps")
            for dc in range(16):
                nc.tensor.matmul(
                    ps[:, :],
                    lhsT=hTb[dc][:, stv * 128:(stv + 1) * 128],
                    rhs=wq[dc][:, 2 * CW:3 * CW],
                    start=(dc == 0), stop=(dc == 15),
                )
            nc.any.tensor_copy(vsb[stv][:, :], ps[:, :])
        # q^T, k^T per head (transposed [feature, token] layout)
        for hh in range(3):
            for ct in (hh, 3 + hh):            # q-tile then k-tile of head hh
                for snb in range(8):
                    ps = psC.tile([128, S_LOC], f32, tag="qk_ps")
                    for dc in range(16):
                        nc.tensor.matmul(
                            ps[:, :],
                            lhsT=wq[dc][:, ct * 128:(ct + 1) * 128],
                            rhs=hTb[dc][:, snb * S_LOC:(snb + 1) * S_LOC],
                            start=(dc == 0), stop=(dc == 15),
                        )
                    nc.any.tensor_copy(qkT[ct][:, snb * S_LOC:(snb + 1) * S_LOC], ps[:, :])

    # ---------------- Phase D: causal attention (3 heads, all tokens) ------
    # kpos-tile PAIRS of 256; exp over [128, 1024] psum; heads interleaved
    with tc.tile_pool(name="phD", bufs=4) as sbD, \
         tc.tile_pool(name="phD_s", bufs=2, space="PSUM") as psDs, \
         tc.tile_pool(name="phD_o", bufs=2, space="PSUM") as psDo, \
         tc.tile_pool(name="phD_r", bufs=1, space="PSUM") as psDr, \
         tc.tile_pool(name="phD_b", bufs=1, space="PSUM") as psDb:
        for qi in range(4):
            for hh in range(3):
                qT = qkT[hh]
                kT = qkT[3 + hh]
                o_ps = psDo.tile([128, 512], f32, tag="o")
                r_ps = psDr.tile([1, 512], f32, tag="r")
                npair = 2 * (qi + 1)
                for kp in range(npair):
                    s_ps = psDs.tile([128, 1024], f32, tag="s")
                    for u in range(2):
                        ki = 2 * kp + u
                        nc.tensor.matmul(
                            s_ps[:, u * 512:(u + 1) * 512],
                            lhsT=kT[:, ki * 128:(ki + 1) * 128],
                            rhs=qT[:, qi * 512:(qi + 1) * 512],
                            start=True, stop=True,
                        )
                    p_sb = sbD.tile([128, 1024], bf16, tag="p")
                    nc.scalar.activation(p_sb[:, :], s_ps[:, :], AF.Exp, scale=SCALE)
                    if kp >= 2 * qi:           # diagonal pair -> causal mask
                        mh = kp - 2 * qi       # 0 or 1
                        nc.vector.tensor_tensor(
                            p_sb[:, :], p_sb[:, :],
                            msk_sb[:, mh * 1024:(mh + 1) * 1024], op=ALU.mult,
                        )
                    for u in range(2):
                        ki = 2 * kp + u
                        nc.tensor.matmul(
                            o_ps[:, :],
                            lhsT=vsb[ki][:, hh * 128:(hh + 1) * 128],
                            rhs=p_sb[:, u * 512:(u + 1) * 512],
                            start=(kp == 0 and u == 0),
                            stop=(kp == npair - 1 and u == 1),
                        )
                        nc.tensor.matmul(
                            r_ps[:, :],
                            lhsT=onc_sb[:, :],
                            rhs=p_sb[:, u * 512:(u + 1) * 512],
                            start=(kp == 0 and u == 0),
                            stop=(kp == npair - 1 and u == 1),
                        )
                rc = sbD.tile([1, 512], f32r, tag="rc")
                with nc.allow_low_precision(reason="f32r broadcast of softmax recip"):
                    nc.vector.reciprocal(rc[:, :], r_ps[:, :])
                bc_ps = psDb.tile([128, 512], f32, tag="bc")
                nc.tensor.matmul(
                    bc_ps[:, :], lhsT=onr_sb[:, :],
                    rhs=rc[:, :], start=True, stop=True,
                )
                bc_sb = sbD.tile([128, 512], f32, tag="bcs")
                nc.vector.tensor_copy(bc_sb[:, :], bc_ps[:, :])
                on_sb = sbD.tile([128, 512], bf16, tag="on")
                nc.vector.tensor_mul(on_sb[:, :], o_ps[:, :], bc_sb[:, :])
                nc.sync.dma_start(
                    a2a_in[:, :].rearrange(
                        "(j c p) t -> c p j t", c=3, p=128)[hh][:, 2 * qi:2 * qi + 2, :],
                    on_sb[:, :].rearrange("p (j t) -> p j t", j=2),
                )

    pCD.release()

    # ---------------- Phase E: AllToAll + Wo + residual --------------------
    nc.gpsimd.collective_compute(
        "AllToAll", ALU.bypass, replica_groups=rg,
        ins=[a2a_in.opt()], outs=[a2a_out.opt()],
    )
    with tc.tile_pool(name="phE", bufs=1) as sbE, \
         tc.tile_pool(name="phE_w", bufs=2) as sbEw, \
         tc.tile_pool(name="phE_ps", bufs=8, space="PSUM") as psE:
        oT = sbE.tile([128, 24 * S_LOC], bf16, name="oT")
        nc.sync.dma_start(
            oT[:, :].rearrange("p (cc j) -> p cc j", j=S_LOC),
            a2a_out[:, :].rearrange("(cc p) j -> p cc j", p=128),
        )
        y_ps = [psE.tile([128, 512], f32, name=f"y_ps{i}", tag="y") for i in range(8)]
        for c4 in range(6):
            wot = sbEw.tile([128, 4 * D], bf16, tag="wo")
            nc.sync.dma_start(
                wot[:, :].rearrange("p (cc d) -> p cc d", cc=4),
                wo[:, :].rearrange("(c4 cc p) d -> c4 p cc d", cc=4, p=128)[c4],
            )
            for cl in range(4):
                cc = c4 * 4 + cl
                for st2 in range(2):
                    for dt4 in range(4):
                        nc.tensor.matmul(
                            y_ps[st2 * 4 + dt4][:, :],
                            lhsT=oT[:, cc * S_LOC + st2 * 128: cc * S_LOC + (st2 + 1) * 128],
                            rhs=wot[:, cl * D + dt4 * 512: cl * D + (dt4 + 1) * 512],
                            start=(cc == 0), stop=(cc == 23),
                        )
        for st2 in range(2):
            for dt4 in range(4):
                nc.vector.scalar_tensor_tensor(
                    out=y_sb[st2][:, dt4 * 512:(dt4 + 1) * 512],
                    in0=y_ps[st2 * 4 + dt4][:, :], scalar=1.0,
                    in1=xln[st2][:, dt4 * 512:(dt4 + 1) * 512],
                    op0=ALU.mult, op1=ALU.add,
                )

    # ---------------- Phase F: y -> y^T ------------------------------------
    pFG = tc.alloc_tile_pool(name="pFG", bufs=1)
    yT = [pFG.tile([128, S_LOC], bf16, name=f"yT{i}") for i in range(16)]
    with tc.tile_pool(name="phF", bufs=2) as sbF, \
         tc.tile_pool(name="phF_ps", bufs=2, space="PSUM") as psF:
        for st2 in range(2):
            ybf = sbF.tile([128, D], bf16, tag="ybf")
            nc.vector.tensor_copy(ybf[:, :], y_sb[st2][:, :])
            for dc in range(16):
                tp = psF.tile([128, 128], bf16, tag="tpf")
                nc.tensor.transpose(tp[:, :], ybf[:, dc * 128:(dc + 1) * 128], ident[:, :])
                nc.vector.tensor_copy(yT[dc][:, st2 * 128:(st2 + 1) * 128], tp[:, :])

    # ---------------- Phase G: FFN1 (gelu(y@W1+b1), transposed out) --------
    pGH = tc.alloc_tile_pool(name="pGH", bufs=1)
    gT = [pGH.tile([128, S_LOC], bf16, name=f"gT{i}") for i in range(64)]
    with tc.tile_pool(name="phG_w", bufs=2) as wg, \
         tc.tile_pool(name="phG_ps", bufs=2, space="PSUM") as psG:
        for fg in range(16):
            w1t = wg.tile([128, 16 * 512], bf16, tag="w1")
            nc.sync.dma_start(
                w1t[:, :].rearrange("p (dc j) -> p dc j", dc=16),
                w1[:, fg * 512:(fg + 1) * 512].rearrange("(dc p) j -> p dc j", p=128),
            )
            for ft in range(4):
                g_ps = psG.tile([128, S_LOC], f32, tag="g")
                for dc in range(16):
                    nc.tensor.matmul(
                        g_ps[:, :],
                        lhsT=w1t[:, dc * 512 + ft * 128: dc * 512 + (ft + 1) * 128],
                        rhs=yT[dc][:, :],
                        start=(dc == 0), stop=(dc == 15),
                    )
                fi = fg * 4 + ft
                nc.scalar.activation(
                    gT[fi][:, :], g_ps[:, :], AF.Gelu_apprx_tanh,
                    bias=b1t_sb[:, fi:fi + 1], scale=1.0,
                )

    # ---------------- Phase H: FFN2 + bias + residual ----------------------
    with tc.tile_pool(name="phH_w", bufs=3) as wh, \
         tc.tile_pool(name="phH_sb", bufs=1) as sbH, \
         tc.tile_pool(name="phH_ps", bufs=4, space="PSUM") as psH:
        out_t = [sbH.tile([128, D], f32, name=f"outsb{i}") for i in range(2)]
        for dt4 in range(4):
            z_ps = [psH.tile([128, 512], f32, tag="z", name=f"z{dt4}_{i}") for i in range(2)]
            for fcg in range(8):
                w2t = wh.tile([128, 8 * 512], bf16, tag="w2")
                nc.sync.dma_start(
                    w2t[:, :].rearrange("p (fc j) -> p fc j", fc=8),
                    w2[:, dt4 * 512:(dt4 + 1) * 512].rearrange(
                        "(fcg fc p) j -> fcg p fc j", fc=8, p=128)[fcg],
                )
                for fl in range(8):
                    fc = fcg * 8 + fl
                    for st2 in range(2):
                        nc.tensor.matmul(
                            z_ps[st2][:, :],
                            lhsT=gT[fc][:, st2 * 128:(st2 + 1) * 128],
                            rhs=w2t[:, fl * 512:(fl + 1) * 512],
                            start=(fc == 0), stop=(fc == 63),
                        )
            for st2 in range(2):
                nc.vector.scalar_tensor_tensor(
                    out=out_t[st2][:, dt4 * 512:(dt4 + 1) * 512],
                    in0=z_ps[st2][:, :], scalar=1.0,
                    in1=y_sb[st2][:, dt4 * 512:(dt4 + 1) * 512],
                    op0=ALU.mult, op1=ALU.add,
                )
                nc.vector.tensor_tensor(
                    out_t[st2][:, dt4 * 512:(dt4 + 1) * 512],
                    out_t[st2][:, dt4 * 512:(dt4 + 1) * 512],
                    b2b_sb[:, dt4 * 512:(dt4 + 1) * 512], op=ALU.add)
        for st2 in range(2):
            nc.sync.dma_start(out_r[st2 * 128:(st2 + 1) * 128, :], out_t[st2][:, :])

    pGH.release()
    pFG.release()
    persist.release()
    constp.release()
    dram.release()


def _build():
    if "nc" in _CACHE:
        return _CACHE["nc"]
    nc = bacc.Bacc("TRN2", target_bir_lowering=False, debug=False,
                   num_devices=N_CORES)

    def I(name, shape, dt):
        return nc.dram_tensor(name, shape, dt, kind="ExternalInput")

    io = (
        I("x_r", [S_LOC, D], f32),
        I("lng", [128, D], bf16),
        I("lnb", [128, D], bf16),
        I("b2b", [128, D], bf16),
        I("b1t", [128, F // 128], f32),
        I("wqkv", [D, 3 * CW], bf16),
        I("wo", [H * Dh, D], bf16),
        I("w1", [D, F], bf16),
        I("w2", [F, D], bf16),
        I("msk", [128, 2048], bf16),
        I("onc", [128, 1], bf16),
        I("onr", [1, 128], f32r),
        nc.dram_tensor("out_r", [S_LOC, D], f32, kind="ExternalOutput"),
    )
    with tile.TileContext(nc) as tc:
        _emit(nc, tc, io)
    nc.compile()
    _CACHE["nc"] = nc
    return nc


def _host_prep(inputs):
    bf = ml_dtypes.bfloat16
    x = np.asarray(inputs["x"], np.float32).reshape(S, D)
    ln_g = np.asarray(inputs["ln_g"], np.float32)
    ln_b = np.asarray(inputs["ln_b"], np.float32)
    attn_g = np.asarray(inputs["attn_g"], np.float32)
    Wq = np.asarray(inputs["Wq"], np.float32)
    Wk = np.asarray(inputs["Wk"], np.float32)
    Wv = np.asarray(inputs["Wv"], np.float32)
    Wo = np.asarray(inputs["Wo"], np.float32)
    W1 = np.asarray(inputs["W1"], np.float32)
    b1 = np.asarray(inputs["b1"], np.float32)
    W2 = np.asarray(inputs["W2"], np.float32)
    b2 = np.asarray(inputs["b2"], np.float32)

    g = attn_g[:, None]
    Wq_s = (Wq * g).astype(bf)
    Wk_s = (Wk * g).astype(bf)
    Wv_s = (Wv * g).astype(bf)
    wo_b = Wo.astype(bf)
    w1_b = W1.astype(bf)
    w2_b = W2.astype(bf)

    lng_b = np.broadcast_to(ln_g[None, :], (128, D)).astype(bf)
    lnb_b = np.broadcast_to(ln_b[None, :], (128, D)).astype(bf)
    b2_b = np.broadcast_to(b2[None, :], (128, D)).astype(bf)
    b1_t = np.ascontiguousarray(b1.reshape(F // 128, 128).T).astype(np.float32)

    i_idx = np.arange(512)[None, :]
    j_idx = np.arange(128)[:, None]
    msk = np.concatenate(
        [(i_idx >= 128 * m + j_idx) for m in range(4)], axis=1
    ).astype(bf)
    onc = np.ones((128, 1), bf)
    onr = np.ones((1, 128), np.float32)

    in_maps = []
    for r in range(N_CORES):
        wqkv_r = np.concatenate(
            [Wq_s[:, r * CW:(r + 1) * CW],
             Wk_s[:, r * CW:(r + 1) * CW],
             Wv_s[:, r * CW:(r + 1) * CW]], axis=1)
        in_maps.append({
            "x_r": np.ascontiguousarray(x[r * S_LOC:(r + 1) * S_LOC, :]),
            "lng": lng_b, "lnb": lnb_b, "b2b": b2_b, "b1t": b1_t,
            "wqkv": np.ascontiguousarray(wqkv_r),
            "wo": wo_b, "w1": w1_b, "w2": w2_b,
            "msk": msk, "onc": onc, "onr": onr,
        })
    return in_maps


def kernel(**inputs) -> np.ndarray:
    nc = _build()
    in_maps = _host_prep(inputs)
    res = run_bass_kernel_spmd(
        nc, in_maps, core_ids=list(range(N_CORES)), trace=TRACE)
    _CACHE["last_result"] = res
    out = np.concatenate([res.results[r]["out_r"] for r in range(N_CORES)], axis=0)
    return out.reshape(1, S, D)
